# revision 59
# baseline (speedup 1.0000x reference)
"""Trainium2 Bass kernel for nn_CaptioningTransformer.

Data-parallel over batch N=8 across the 8 NeuronCores (one caption per core).
Each core runs the full 2-layer decoder + the (512,512)@(512,32000) logits
projection for its caption. Layer matmuls run in bf16 (fp32 PSUM
accumulation); LayerNorm / softmax statistics / residual stream stay fp32.

The logits projection runs in fp8e4 DoubleRow mode with a 3-term hi/lo
decomposition: x ~ (xh + xl)/SX, w ~ (wh + wl)/SW, and
x@w ~ (xh@wh + xh@wl + xl@wh) / (SX*SW), all three products accumulated in
one PSUM group (shared scale). Residuals stay in e4m3 normal range thanks to
the large base scales, so precision matches bf16. Logits are written to HBM
as bf16 and upcast on the host.

Self-contained: hardcodes all shapes; takes FULL inputs, returns FULL output.
"""

import math
from contextlib import ExitStack

import ml_dtypes
import numpy as np

import concourse.bacc as bacc
import concourse.bass as bass
import concourse.tile as tile
from concourse import mybir
from concourse.bass_utils import run_bass_kernel_spmd
from concourse.masks import make_causal_mask, make_identity

# dims
N, T, D, W, H, V, L, FF = 8, 512, 1024, 512, 4, 32000, 2, 2048
P = 128
TC = T // P            # 4 token chunks
KC = W // P            # 4 feature chunks
DC = D // P            # 8 vis-feature chunks
FFC = FF // P          # 16 ffn chunks
HD = W // H            # 128 head dim (== P)
VG = 2000              # vocab columns per DMA group
NVG = V // VG          # 16 groups
SV = 500               # vocab columns per psum tile
NSV = VG // SV         # 4 subtiles per group
EPS = 1e-5
SCALE = 1.0 / math.sqrt(HD)
CPACK_COLS = 4 + DC + 4 * L + 4 * L + 4 * L + FFC * L + W * L

F32 = mybir.dt.float32
BF16 = mybir.dt.bfloat16
F8 = mybir.dt.float8e4
I32 = mybir.dt.int32
AX = mybir.AxisListType
ALU = mybir.AluOpType
ACTF = mybir.ActivationFunctionType
PERF2 = mybir.MatmulPerfMode.DoubleRow
BF16_NP = ml_dtypes.bfloat16
F8_NP = ml_dtypes.float8_e4m3

# fp8 hi/lo base scales for the logits matmul (shared-scale 3-term scheme)
SX = 32.0
SW = 512.0
OSCALE = 1.0 / (SX * SW)


def _wrap_p(a, np_dtype):
    """[..., k*P, X] -> [..., P, k, X] (partition-major wrap of the -2 axis)."""
    a = np.asarray(a)
    lead = a.shape[:-2]
    k = a.shape[-2] // P
    x = a.shape[-1]
    a = a.reshape(*lead, k, P, x)
    a = np.moveaxis(a, -2, -3)  # [..., P, k, x]
    return np.ascontiguousarray(a.astype(np_dtype))


def _wrap_vec(v, np_dtype):
    """[..., k*P] -> [..., P, k]."""
    v = np.asarray(v)
    lead = v.shape[:-1]
    k = v.shape[-1] // P
    v = v.reshape(*lead, k, P)
    v = np.moveaxis(v, -1, -2)
    return np.ascontiguousarray(v.astype(np_dtype))


def _build(row_biases_zero: bool, ln_trivial: bool, stop_after: str | None = None):
    nc = bacc.Bacc(
        "TRN2", target_bir_lowering=False, debug=False, enable_asserts=False
    )

    def din(name, shape, dt):
        return nc.dram_tensor(name, list(shape), dt, kind="ExternalInput").ap()

    # ---- DRAM inputs (per core) ----
    x0_d = din("x0", [P, TC, W], F32)             # emb[captions] + pe (host)
    visw_d = din("visw", [P, DC, W], BF16)
    # packed f32 consts: visb(4) feat(8) sabq(2*4) sabk(2*4) cabv(2*4)
    # ff1b(2*16) then cabo rows (row 0 only, 2*512)
    cpack_d = din("cpack", [P, CPACK_COLS], F32)
    sa_d = din("sa", [L, P, 4, KC, W], BF16)      # q,k,v,o packed
    ca_d = din("ca", [L, P, 2, KC, W], BF16)      # wv,wo packed
    ff_d = din("ff", [L, P, 2, KC * FF], BF16)    # ff1 flat, ff2 flat
    whi_d = din("whi", [P, KC, V], F8)            # e4m3(out_w * SW) hi part
    wlo_d = din("wlo", [P, KC, V], F8)            # e4m3 residual (same scale)
    if not row_biases_zero:
        sabv_d = din("sabv", [L, 1, W], BF16)
        sabo_d = din("sabo", [L, 1, W], BF16)
        ff2b_d = din("ff2b", [L, 1, W], BF16)
        outb_d = din("outb", [1, V], BF16)
    if not ln_trivial:
        lnw_d = [din(f"ln{i}w", [L, 1, W], F32) for i in (1, 2, 3)]
        lnb_d = [din(f"ln{i}b", [L, 1, W], F32) for i in (1, 2, 3)]

    out_d = nc.dram_tensor("logits", [T, V], BF16, kind="ExternalOutput").ap()

    with tile.TileContext(nc) as tc, ExitStack() as ctx:
        consts = ctx.enter_context(tc.tile_pool(name="consts", bufs=1))
        xpool = ctx.enter_context(tc.tile_pool(name="xpool", bufs=1))
        wpool = ctx.enter_context(tc.tile_pool(name="wpool", bufs=1))
        work = ctx.enter_context(tc.tile_pool(name="work", bufs=1))
        hot = ctx.enter_context(tc.tile_pool(name="hot", bufs=3))
        sap = ctx.enter_context(tc.tile_pool(name="sap", bufs=2))
        # fallback variants carry extra bias/LN tiles; shrink the perf-only
        # rings there (that path's speed is irrelevant)
        _fast = row_biases_zero and ln_trivial
        wpre = ctx.enter_context(tc.tile_pool(name="wpre", bufs=2 if _fast else 1))
        ostp = ctx.enter_context(tc.tile_pool(name="ostp", bufs=6 if _fast else 2))
        if not ln_trivial:
            lnp = ctx.enter_context(tc.tile_pool(name="lnp", bufs=1))
            # 12 broadcast tiles do not fit; keep one (w, b) pair per ln
            # index resident and reload per layer inside the ring

        psA = ctx.enter_context(tc.tile_pool(name="psA", bufs=2, space="PSUM"))
        psS = ctx.enter_context(tc.tile_pool(name="psS", bufs=3, space="PSUM"))
        psT = ctx.enter_context(tc.tile_pool(name="psT", bufs=3, space="PSUM"))

        # ---- residual stream: x0 = emb[captions] + pe, gathered host-side;
        # loaded per chunk so the first transpose starts as early as possible
        x_sb = xpool.tile([P, TC, W], F32)
        for c in range(TC):
            nc.sync.dma_start(x_sb[:, c], x0_d[:, c])

        # ---- constants ----
        ident_f32 = consts.tile([P, P], F32)
        make_identity(nc, ident_f32[:])
        causalT = consts.tile([P, P], F32)
        nc.gpsimd.memset(causalT[:], 0.0)
        nc.gpsimd.affine_select(
            out=causalT[:], in_=causalT[:], compare_op=ALU.is_ge,
            fill=-1e9, base=0, pattern=[[1, P]], channel_multiplier=-1,
        )
        ones_col_bf = consts.tile([P, 1], BF16)
        nc.vector.memset(ones_col_bf[:], 1.0)
        ones_bf = consts.tile([1, P], BF16)
        nc.vector.memset(ones_bf[:], 1.0)
        ones_f32 = consts.tile([1, P], F32)
        nc.vector.memset(ones_f32[:], 1.0)
        eps_sb = consts.tile([P, 1], F32)
        nc.vector.memset(eps_sb[:], EPS)

        cpack_sb = consts.tile([P, CPACK_COLS], F32)
        nc.sync.dma_start(cpack_sb[:], cpack_d[:])
        o = 0
        visb_sb = cpack_sb[:, o : o + KC]; o += KC
        feat_sb = cpack_sb[:, o : o + DC]; o += DC
        sabq_sb = [cpack_sb[:, o + 4 * l : o + 4 * (l + 1)] for l in range(L)]
        o += 4 * L
        sabk_sb = [cpack_sb[:, o + 4 * l : o + 4 * (l + 1)] for l in range(L)]
        o += 4 * L
        cabv_sb = [cpack_sb[:, o + 4 * l : o + 4 * (l + 1)] for l in range(L)]
        o += 4 * L
        ff1b_sb = [cpack_sb[:, o + FFC * l : o + FFC * (l + 1)] for l in range(L)]
        o += FFC * L
        cabo_sb = [cpack_sb[0:1, o + W * l : o + W * (l + 1)] for l in range(L)]
        o += W * L
        featb_sb = consts.tile([P, DC], BF16)
        nc.vector.tensor_copy(featb_sb[:], feat_sb)

        def per_layer_rows(dram, nm, dt, shape):
            tiles = []
            for l in range(L):
                t = consts.tile(shape, dt, name=f"{nm}{l}")
                nc.sync.dma_start(t[:], dram[l])
                tiles.append(t)
            return tiles
        if not row_biases_zero:
            sabv_sb = per_layer_rows(sabv_d, "sabv", BF16, [1, W])
            sabo_sb = per_layer_rows(sabo_d, "sabo", BF16, [1, W])
            ff2b_sb = per_layer_rows(ff2b_d, "ff2b", BF16, [1, W])
            # out_b enters the fp8 logits PSUM at the shared pre-scale
            ones16k_bf = consts.tile([1, P], BF16)
            nc.vector.memset(ones16k_bf[:], 1.0 / OSCALE)
        if not ln_trivial:
            # broadcast ln scale/bias rows across partitions once
            lnw_bc = [[None] * L for _ in range(3)]
            lnb_bc = [[None] * L for _ in range(3)]
            for i in range(3):
                for l in range(L):
                    wt = lnp.tile([P, W], F32, name=f"lnwbc{i}_{l}", tag=f"lnw{i}", bufs=1)
                    nc.gpsimd.dma_start(wt[:], lnw_d[i][l].to_broadcast([P, W]))
                    lnw_bc[i][l] = wt
                    bt = lnp.tile([P, W], F32, name=f"lnbbc{i}_{l}", tag=f"lnb{i}", bufs=1)
                    nc.gpsimd.dma_start(bt[:], lnb_d[i][l].to_broadcast([P, W]))
                    lnb_bc[i][l] = bt

        # ---- layer-0 self-attention weights; q,k first (critical path) ----
        sa0_sb = sap.tile([P, 4, KC, W], BF16, name="sa_sb", tag="sa_sb")
        for q in range(2):
            nc.sync.dma_start(sa0_sb[:, q], sa_d[0, :, q])

        # ---- vis projection weights ----
        visw_sb = work.tile([P, DC, W], BF16, name="visw_sb", tag="hT")
        nc.sync.dma_start(visw_sb[:], visw_d[:])
        for q in range(2, 4):
            nc.sync.dma_start(sa0_sb[:, q], sa_d[0, :, q])

        _stages = {
            "embed": 0, "memT": 1, "sa0": 2, "ca0": 3, "l0": 4, "l1": 5,
            "logits1": 6, None: 99,
        }
        srank = _stages[stop_after]

        # cross-attention weights: loaded upfront (DMA overlaps layer-0 SA),
        # consumed by the deferred precompute below
        ca_sb_tiles = []
        if srank >= 3:
            for l in range(L):
                cal_sb = wpool.tile([P, 2, KC, W], BF16, name=f"ca{l}", tag=f"ca{l}")
                nc.sync.dma_start(cal_sb[:, 0], ca_d[l, :, 0])
                nc.sync.dma_start(cal_sb[:, 1], ca_d[l, :, 1])
                ca_sb_tiles.append(cal_sb)

        # ---- memory vector memT = (features @ vis_w + vis_b), transposed [W,1]
        # Runs between layer-0 SA and the first ca-add (off the startup
        # critical path; the PE is otherwise busy with attention by then).
        memT_sb = consts.tile([P, KC], BF16)

        def precompute_memT():
            for o in range(KC):
                pm = psS.tile([P, 512], F32, name="psS", tag="psS")
                for ki in range(DC):
                    nc.tensor.matmul(
                        pm[:, :1],
                        lhsT=visw_sb[:, ki, o * P : (o + 1) * P],
                        rhs=featb_sb[:, ki : ki + 1],
                        start=(ki == 0),
                        stop=(ki == DC - 1),
                    )
                nc.scalar.activation(
                    memT_sb[:, o : o + 1], pm[:, :1], ACTF.Identity,
                    bias=visb_sb[:, o : o + 1], scale=1.0,
                )

        # ---- cross-attention rows (x-independent: softmax over single key
        # is identically 1, so ca_out = (mem@wv+bv)@wo+bo broadcast over T).
        ca_bc = []

        def precompute_ca():
            precompute_memT()
            for l in range(L):
                cawv_sb, cawo_sb = ca_sb_tiles[l][:, 0], ca_sb_tiles[l][:, 1]
                vTca = hot.tile([P, KC], BF16, name="vTca", tag="vTca")
                for o in range(KC):
                    pm = psS.tile([P, 512], F32, name="psS", tag="psS")
                    for ki in range(KC):
                        nc.tensor.matmul(
                            pm[:, :1],
                            lhsT=cawv_sb[:, ki, o * P : (o + 1) * P],
                            rhs=memT_sb[:, ki : ki + 1],
                            start=(ki == 0),
                            stop=(ki == KC - 1),
                        )
                    nc.scalar.activation(
                        vTca[:, o : o + 1], pm[:, :1], ACTF.Identity,
                        bias=cabv_sb[l][:, o : o + 1], scale=1.0,
                    )
                pr = psS.tile([P, 512], F32, name="psS", tag="psS")
                for o in range(KC):
                    nc.tensor.matmul(
                        pr[:1, :],
                        lhsT=vTca[:, o : o + 1],
                        rhs=cawo_sb[:, o, :],
                        start=(o == 0),
                        stop=(o == KC - 1),
                    )
                ca_row = hot.tile([1, W], F32, name="ca_row", tag="ca_row", bufs=1)
                nc.vector.tensor_tensor(
                    ca_row[:], pr[:1, :], cabo_sb[l], op=ALU.add
                )
                pbc = psS.tile([P, 512], F32, name="psS", tag="psS")
                nc.tensor.matmul(
                    pbc[:], lhsT=ones_f32[:], rhs=ca_row[:], start=True, stop=True
                )
                cb = consts.tile([P, W], F32, name=f"ca_bc{l}")
                nc.scalar.copy(cb[:], pbc[:])
                ca_bc.append(cb)

        def ln_chunk(ln_idx, l, c):
            """x_sb[:, c] <- LN(x_sb[:, c]) (free-axis stats)."""
            if True:
                stats = hot.tile([P, 6], F32, name="lnstats", tag="lnstats")
                nc.vector.bn_stats(stats[:], x_sb[:, c, :])
                mv = hot.tile([P, 2], F32, name="lnmv", tag="lnmv")
                nc.vector.bn_aggr(mv[:], stats[:])
                std = hot.tile([P, 1], F32, name="lnstd", tag="lnstd")
                nc.scalar.activation(
                    std[:], mv[:, 1:2], ACTF.Sqrt, bias=eps_sb[:], scale=1.0
                )
                rstd = hot.tile([P, 1], F32, name="lnrstd", tag="lnrstd")
                nc.vector.reciprocal(rstd[:], std[:])
                nmr = hot.tile([P, 1], F32, name="lnnmr", tag="lnnmr")
                nc.vector.scalar_tensor_tensor(
                    nmr[:], mv[:, 0:1], -1.0, rstd[:],
                    op0=ALU.mult, op1=ALU.mult,
                )
                nc.scalar.activation(
                    x_sb[:, c, :], x_sb[:, c, :], ACTF.Identity,
                    bias=nmr[:], scale=rstd[:],
                )
                if not ln_trivial:
                    nc.vector.tensor_tensor(
                        x_sb[:, c, :], x_sb[:, c, :], lnw_bc[ln_idx][l][:],
                        op=ALU.mult,
                    )
                    nc.vector.tensor_tensor(
                        x_sb[:, c, :], x_sb[:, c, :], lnb_bc[ln_idx][l][:],
                        op=ALU.add,
                    )

        def transpose_chunk(xt_tile, c):
            """xt_tile[p, o, c*P:(c+1)*P] <- x_sb[:, c, :].T (bf16)"""
            for o in range(KC):
                pt = psT.tile([P, P], F32, name="ptr", tag="ptr")
                nc.tensor.transpose(
                    pt[:], x_sb[:, c, o * P : (o + 1) * P], ident_f32[:]
                )
                if (c + o) % 2 == 0:
                    nc.vector.tensor_copy(
                        xt_tile[:, o, c * P : (c + 1) * P], pt[:]
                    )
                else:
                    nc.scalar.copy(xt_tile[:, o, c * P : (c + 1) * P], pt[:])

        def transpose_x_to(xt_tile):
            """xt_tile[p, o, t] (bf16) <- x_sb[t%P, t//P, o*P+p]"""
            for c in range(TC):
                for o in range(KC):
                    pt = psT.tile([P, P], F32, name="ptr", tag="ptr")
                    nc.tensor.transpose(
                        pt[:], x_sb[:, c, o * P : (o + 1) * P], ident_f32[:]
                    )
                    if (c + o) % 2 == 0:
                        nc.vector.tensor_copy(
                            xt_tile[:, o, c * P : (c + 1) * P], pt[:]
                        )
                    else:
                        nc.scalar.copy(xt_tile[:, o, c * P : (c + 1) * P], pt[:])

        # logits-weight prefetch ring: [hi, lo] pair per vocab group, issued
        # early on the SP queue so transfers fill DMA slack under layer compute
        wl_tiles = {}

        def prefetch_wl(vg, eng=None):
            if vg >= NVG or srank < 5 or vg in wl_tiles:
                return
            # rotate through the wpre ring plus the sa/ff weight buffers
            # (dead after their layer) for extra prefetch depth
            slot = vg % 4 if vg >= 4 else vg
            if slot < 2:
                t = wpre.tile([P, 2, KC, VG], F8, name="wl", tag="wl")
            elif slot == 2:
                t = sap.tile([P, 2, KC, VG], F8, name="wl", tag="sa_sb")
            else:
                t = wpool.tile([P, 2, KC, VG], F8, name="wl", tag="ff_sb")
            # layer-phase prefetches issue on SP (ordered after the layer
            # weight loads); steady-state ones on the idle Pool queue so the
            # ring-slot wait does not block SP's latency-critical ost writes
            eng = eng or nc.gpsimd
            eng.dma_start(t[:, 0], whi_d[:, :, vg * VG : (vg + 1) * VG])
            eng.dma_start(t[:, 1], wlo_d[:, :, vg * VG : (vg + 1) * VG])
            wl_tiles[vg] = t

        # logits operand: xh8 = e4m3(SX*x), xl8 = e4m3(SX*x - xh8), [W, T].
        # Produced chunk-by-chunk inside the final layer norm.
        xh8 = work.tile([P, KC, T], F8, name="xh8", tag="qT")
        xl8 = work.tile([P, KC, T], F8, name="xl8", tag="kT")

        def convert_chunk(c):
            for o in range(KC):
                pt = psT.tile([P, P], F32, name="ptr", tag="ptr")
                nc.tensor.transpose(
                    pt[:], x_sb[:, c, o * P : (o + 1) * P], ident_f32[:]
                )
                hs = xh8[:, o, c * P : (c + 1) * P]
                nc.scalar.activation(hs, pt[:], ACTF.Copy, scale=SX)
                nc.vector.scalar_tensor_tensor(
                    xl8[:, o, c * P : (c + 1) * P], pt[:], SX, hs,
                    op0=ALU.mult, op1=ALU.subtract,
                )

        # ================= layers =================
        for l in range(L if srank >= 2 else 0):
            # ---- self attention ----
            if l == 0:
                sal_sb = sa0_sb
            else:
                sal_sb = sap.tile([P, 4, KC, W], BF16, name="sa_sb", tag="sa_sb")
                for q in range(4):
                    nc.sync.dma_start(sal_sb[:, q], sa_d[l, :, q])
                prefetch_wl(2, nc.sync)
            saq_sb, sak_sb = sal_sb[:, 0], sal_sb[:, 1]
            sav_sb, sao_sb = sal_sb[:, 2], sal_sb[:, 3]

            if l == 0:
                xT = work.tile([P, KC, T], BF16, name="xT", tag="xT")
                transpose_x_to(xT)
            else:
                xT = xT_next

            qT = work.tile([P, KC, T], BF16, name="qT", tag="qT")
            kT = work.tile([P, KC, T], BF16, name="kT", tag="kT")
            qk_cw = P
            for dst, wsb, bsb in ((qT, saq_sb, sabq_sb[l]), (kT, sak_sb, sabk_sb[l])):
                for o in range(KC):
                    pq = psA.tile([P, 512], F32, name="psA", tag="psA")
                    for t0 in range(0, T, qk_cw):
                        for ki in range(KC):
                            nc.tensor.matmul(
                                pq[:, t0 : t0 + qk_cw],
                                lhsT=wsb[:, ki, o * P : (o + 1) * P],
                                rhs=xT[:, ki, t0 : t0 + qk_cw],
                                start=(ki == 0),
                                stop=(ki == KC - 1),
                            )
                    nc.scalar.activation(
                        dst[:, o, :], pq[:], ACTF.Identity,
                        bias=bsb[:, o : o + 1], scale=1.0,
                    )
            v_sb = work.tile([P, TC, W], BF16, name="v_sb", tag="v_sb")
            for c in range(TC):
                pv = psA.tile([P, 512], F32, name="psA", tag="psA")
                first = True
                if not row_biases_zero:
                    nc.tensor.matmul(
                        pv[:], lhsT=ones_bf[:], rhs=sabv_sb[l][:],
                        start=True, stop=False,
                    )
                    first = False
                for ki in range(KC):
                    nc.tensor.matmul(
                        pv[:],
                        lhsT=xT[:, ki, c * P : (c + 1) * P],
                        rhs=sav_sb[:, ki, :],
                        start=first,
                        stop=(ki == KC - 1),
                    )
                    first = False
                nc.vector.tensor_copy(v_sb[:, c, :], pv[:])

            yT = work.tile([P, H, T], BF16, name="yT", tag="yT")
            for h in range(H):
                # scores computed pre-transposed [tk, tq] (swap q/k roles), so
                # exp() writes the A@V operand directly -- no PE transposes.
                # Probs stay UNNORMALIZED (exp can't overflow at these scales);
                # normalization is applied per-head at the out-projection.
                AT = work.tile([P, TC, T], BF16, name="AT", tag="AT", bufs=3)
                for j in range(TC):
                    nv = T - j * P  # valid tq suffix for tk-chunk j
                    ps = psS.tile([P, 512], F32, name="psS", tag="psS")
                    nc.tensor.matmul(
                        ps[:, :nv],
                        lhsT=kT[:, h, j * P : (j + 1) * P],
                        rhs=qT[:, h, j * P :],
                        start=True,
                        stop=True,
                    )
                    # additive -1e9 strict lower-triangle on the diagonal block
                    nc.vector.tensor_tensor(
                        ps[:, :P], ps[:, :P], causalT[:], op=ALU.add
                    )
                    nc.scalar.activation(
                        AT[:, j, j * P :], ps[:, :nv], ACTF.Exp,
                        bias=0.0, scale=SCALE,
                    )
                # per-tq row sums of the unnormalized probs as a [1, T] row
                # (ones^T @ AT); the reciprocal row is broadcast back over
                # partitions with one ones-matmul, overlapping the AV matmuls
                rsum = psT.tile([P, 512], F32, name="rsum", tag="ptr")
                for j in range(TC):
                    nc.tensor.matmul(
                        rsum[:1, j * P :],
                        lhsT=ones_col_bf[:],
                        rhs=AT[:, j, j * P :],
                        start=(j == 0),
                        stop=(j == TC - 1),
                    )
                rrow = hot.tile([1, T], BF16, name="rrow", tag="rrow")
                with nc.allow_low_precision("probs are unnormalized O(1)"):
                    nc.vector.reciprocal(rrow[:], rsum[:1, :])
                py = psA.tile([P, 512], F32, name="psY", tag="psA")
                for j in range(TC):
                    nc.tensor.matmul(
                        py[:, j * P :],
                        lhsT=v_sb[:, j, h * HD : (h + 1) * HD],
                        rhs=AT[:, j, j * P :],
                        start=(j == 0),
                        stop=(j == TC - 1),
                    )
                # pb[d, tok] = 1/rowsum broadcast over the d-partitions;
                # yT = py * pb (normalized here so the out-projection can
                # accumulate all heads in one PSUM group)
                pb = psA.tile([P, 512], F32, name="pb", tag="psA")
                nc.tensor.matmul(
                    pb[:], lhsT=ones_bf[:], rhs=rrow[:], start=True, stop=True
                )
                # DVE can read only one PSUM operand; stage pb through SBUF
                pb_sb = hot.tile([P, T], BF16, name="pb_sb", tag="pb_sb", bufs=2)
                nc.scalar.copy(pb_sb[:], pb[:])
                nc.vector.tensor_tensor(yT[:, h, :], py[:], pb_sb[:], op=ALU.mult)

            # per-head out projection; normalization folded into the
            # per-partition scale of the fused residual accumulate
            if l == 0 and srank >= 3:
                precompute_ca()
            if srank >= 4:
                xT2 = work.tile([P, KC, T], BF16, name="xT2", tag="xT")
            for c in range(TC):
                po = psT.tile([P, 512], F32, name="po", tag="ptr")
                for h in range(H):
                    nc.tensor.matmul(
                        po[:],
                        lhsT=yT[:, h, c * P : (c + 1) * P],
                        rhs=sao_sb[:, h, :],
                        start=(h == 0),
                        stop=(h == H - 1),
                    )
                nc.vector.tensor_add(x_sb[:, c, :], x_sb[:, c, :], po[:])
                if not row_biases_zero:
                    pob = psS.tile([P, 512], F32, name="psS", tag="psS")
                    nc.tensor.matmul(
                        pob[:], lhsT=ones_bf[:], rhs=sabo_sb[l][:],
                        start=True, stop=True,
                    )
                    nc.vector.tensor_add(x_sb[:, c, :], x_sb[:, c, :], pob[:])
                ln_chunk(0, l, c)
            if srank == 2:
                break
            for c in range(TC):
                nc.vector.tensor_add(x_sb[:, c, :], x_sb[:, c, :], ca_bc[l][:])
                ln_chunk(1, l, c)
            for c in range(TC):
                if srank >= 4:
                    transpose_chunk(xT2, c)
            if srank == 3:
                break

            # ---- ffn ----
            ffl_sb = wpool.tile([P, 2, KC * FF], BF16, name="ff_sb", tag="ff_sb")
            nc.sync.dma_start(ffl_sb[:, 0], ff_d[l, :, 0])
            nc.sync.dma_start(ffl_sb[:, 1], ff_d[l, :, 1])
            if l == 0:
                for pv in range(2):
                    prefetch_wl(pv, nc.sync)
            else:
                prefetch_wl(3, nc.sync)
            ff1_sb = ffl_sb[:, 0].rearrange("p (k f) -> p k f", k=KC)
            ff2_sb = ffl_sb[:, 1].rearrange("p (m w) -> p m w", m=FFC)

            hT = work.tile([P, FFC, T], BF16, name="hT", tag="hT")
            for m in range(FFC):
                ph = psA.tile([P, 512], F32, name="psA", tag="psA")
                for t0 in range(0, T, P):
                    for ki in range(KC):
                        nc.tensor.matmul(
                            ph[:, t0 : t0 + P],
                            lhsT=ff1_sb[:, ki, m * P : (m + 1) * P],
                            rhs=xT2[:, ki, t0 : t0 + P],
                            start=(ki == 0),
                            stop=(ki == KC - 1),
                        )
                nc.scalar.activation(
                    hT[:, m, :], ph[:], ACTF.Relu,
                    bias=ff1b_sb[l][:, m : m + 1], scale=1.0,
                )
            if l < L - 1:
                xT_next = work.tile([P, KC, T], BF16, name="xT", tag="xT")
            for c in range(TC):
                pf2 = psA.tile([P, 512], F32, name="psA", tag="psA")
                first = True
                if not row_biases_zero:
                    nc.tensor.matmul(
                        pf2[:], lhsT=ones_bf[:], rhs=ff2b_sb[l][:],
                        start=True, stop=False,
                    )
                    first = False
                for m in range(FFC):
                    nc.tensor.matmul(
                        pf2[:],
                        lhsT=hT[:, m, c * P : (c + 1) * P],
                        rhs=ff2_sb[:, m, :],
                        start=first,
                        stop=(m == FFC - 1),
                    )
                    first = False
                nc.vector.tensor_add(x_sb[:, c, :], x_sb[:, c, :], pf2[:])
                ln_chunk(2, l, c)
                if l == L - 1 and srank >= 5:
                    convert_chunk(c)
                elif l < L - 1:
                    transpose_chunk(xT_next, c)
            if srank == 4:
                break

        # ================= logits (fp8 3-term DoubleRow) =================
        _nvg = NVG if srank >= 99 else (1 if srank >= 6 else 0)
        pcnt = 0
        for vg in range(_nvg):
            if vg not in wl_tiles:
                prefetch_wl(vg)
            wl = wl_tiles.pop(vg)
            if not row_biases_zero:
                ob_sb = ostp.tile([1, VG], BF16, name="ob", tag="ob", bufs=2)
                nc.sync.dma_start(ob_sb[:], outb_d[:, vg * VG : (vg + 1) * VG])
            for c in range(TC):
                ost = ostp.tile([P, VG], BF16, name="ost", tag="ost")
                for sv in range(NSV):
                    plp = (psA, psS, psT)[pcnt % 3]
                    pcnt += 1
                    ptag = "ptr" if plp is psT else plp.name
                    pl = plp.tile([P, 512], F32, name="psL", tag=ptag)
                    first = True
                    if not row_biases_zero:
                        nc.tensor.matmul(
                            pl[:, :SV],
                            lhsT=ones16k_bf[:],
                            rhs=ob_sb[:, sv * SV : (sv + 1) * SV],
                            start=True,
                            stop=False,
                            skip_group_check=True,
                        )
                        first = False
                    svw = slice(sv * SV, (sv + 1) * SV)
                    for xt, wt in ((xh8, 0), (xh8, 1), (xl8, 0)):
                        for kk in (0, 2):
                            nc.tensor.matmul(
                                pl[:, :SV],
                                lhsT=xt[:, kk : kk + 2, c * P : (c + 1) * P],
                                rhs=wl[:, wt, kk : kk + 2, svw],
                                start=first,
                                stop=(xt is xl8 and kk == 2),
                                perf_mode=PERF2,
                                skip_group_check=not row_biases_zero,
                            )
                            first = False
                    if pcnt % 2 == 0:
                        nc.vector.tensor_scalar_mul(ost[:, svw], pl[:, :SV], OSCALE)
                    else:
                        nc.scalar.activation(
                            ost[:, svw], pl[:, :SV], ACTF.Copy, scale=OSCALE
                        )
                nc.sync.dma_start(
                    out_d[c * P : (c + 1) * P, vg * VG : (vg + 1) * VG], ost[:]
                )
            prefetch_wl(vg + 4)

        if stop_after is not None:
            xdbg = nc.dram_tensor(
                "xdbg", [P, TC, W], F32, kind="ExternalOutput"
            ).ap()
            nc.sync.dma_start(xdbg[:], x_sb[:])

    nc.compile()
    return nc


_BUILD_CACHE = {}


def _get_nc(row_biases_zero, ln_trivial):
    key = (row_biases_zero, ln_trivial)
    if key not in _BUILD_CACHE:
        _BUILD_CACHE[key] = _build(*key)
    return _BUILD_CACHE[key]


def _prep_in_maps(inputs):
    f32 = np.float32
    features = np.asarray(inputs["features"], f32)          # (N, D)
    captions = np.asarray(inputs["captions"])               # (N, T) int
    emb = np.asarray(inputs["emb"], f32)                    # (V, W)
    pe = np.asarray(inputs["pe"], f32)                      # (T, W)
    x0 = emb[captions] + pe[None]                           # (N, T, W)

    row_biases_zero = all(
        not np.any(np.asarray(inputs[k]))
        for k in ("sa_bv", "sa_bo", "ff2_b", "out_b")
    )
    ln_trivial = all(
        np.all(np.asarray(inputs[f"ln{i}_w"]) == 1.0)
        and not np.any(np.asarray(inputs[f"ln{i}_b"]))
        for i in (1, 2, 3)
    )

    sa_pack = np.stack(
        [_wrap_p(np.asarray(inputs[k]), BF16_NP)
         for k in ("sa_wq", "sa_wk", "sa_wv", "sa_wo")], axis=1
    )  # [L, 4, P, KC, W] -> want [L, P, 4, KC, W]
    sa_pack = np.ascontiguousarray(np.moveaxis(sa_pack, 1, 2))
    ca_pack = np.stack(
        [_wrap_p(np.asarray(inputs[k]), BF16_NP) for k in ("ca_wv", "ca_wo")],
        axis=1,
    )
    ca_pack = np.ascontiguousarray(np.moveaxis(ca_pack, 1, 2))
    ff1w = _wrap_p(np.asarray(inputs["ff1_w"]), BF16_NP).reshape(L, P, KC * FF)
    ff2w = _wrap_p(np.asarray(inputs["ff2_w"]), BF16_NP).reshape(L, P, FFC * W)
    ff_pack = np.ascontiguousarray(np.stack([ff1w, ff2w], axis=2))  # [L,P,2,KF]

    cpack = np.zeros((P, CPACK_COLS), f32)
    o = 0
    cpack[:, o : o + KC] = _wrap_vec(np.asarray(inputs["vis_b"]), f32); o += KC
    feat_off = o; o += DC  # per-core features slot
    sabq = _wrap_vec(np.asarray(inputs["sa_bq"]), f32)
    sabk = _wrap_vec(np.asarray(inputs["sa_bk"]), f32)
    cabv = _wrap_vec(np.asarray(inputs["ca_bv"]), f32)
    ff1b = _wrap_vec(np.asarray(inputs["ff1_b"]), f32)
    cabo = np.asarray(inputs["ca_bo"], f32)
    for l in range(L):
        cpack[:, o + 4 * l : o + 4 * (l + 1)] = sabq[l]
    o += 4 * L
    for l in range(L):
        cpack[:, o + 4 * l : o + 4 * (l + 1)] = sabk[l]
    o += 4 * L
    for l in range(L):
        cpack[:, o + 4 * l : o + 4 * (l + 1)] = cabv[l]
    o += 4 * L
    for l in range(L):
        cpack[:, o + FFC * l : o + FFC * (l + 1)] = ff1b[l]
    o += FFC * L
    for l in range(L):
        cpack[0, o + W * l : o + W * (l + 1)] = cabo[l]
    o += W * L
    assert o == CPACK_COLS

    # fp8 hi/lo split of out_w at shared scale SW; [W, V] -> [P, KC, V]
    w_s = np.asarray(inputs["out_w"], f32) * SW
    whi = np.clip(w_s, -240.0, 240.0).astype(F8_NP)
    wlo = np.clip(w_s - whi.astype(f32), -240.0, 240.0).astype(F8_NP)

    def _wrap_w(a):
        a = a.reshape(KC, P, V)
        return np.ascontiguousarray(np.moveaxis(a, 0, 1))

    shared = {
        "visw": _wrap_p(np.asarray(inputs["vis_w"]), BF16_NP),
        "sa": sa_pack,
        "ca": ca_pack,
        "ff": ff_pack,
        "whi": _wrap_w(whi),
        "wlo": _wrap_w(wlo),
    }
    if not row_biases_zero:
        shared["sabv"] = np.ascontiguousarray(
            np.asarray(inputs["sa_bv"]).astype(BF16_NP).reshape(L, 1, W)
        )
        shared["sabo"] = np.ascontiguousarray(
            np.asarray(inputs["sa_bo"]).astype(BF16_NP).reshape(L, 1, W)
        )
        shared["ff2b"] = np.ascontiguousarray(
            np.asarray(inputs["ff2_b"]).astype(BF16_NP).reshape(L, 1, W)
        )
        shared["outb"] = np.ascontiguousarray(
            np.asarray(inputs["out_b"]).astype(BF16_NP).reshape(1, V)
        )
    if not ln_trivial:
        for i in (1, 2, 3):
            shared[f"ln{i}w"] = np.ascontiguousarray(
                np.asarray(inputs[f"ln{i}_w"], f32).reshape(L, 1, W)
            )
            shared[f"ln{i}b"] = np.ascontiguousarray(
                np.asarray(inputs[f"ln{i}_b"], f32).reshape(L, 1, W)
            )

    in_maps = []
    for i in range(N):
        m = dict(shared)
        m["x0"] = _wrap_p(x0[i], f32)
        cp = cpack.copy()
        cp[:, feat_off : feat_off + DC] = features[i].reshape(DC, P).T
        m["cpack"] = cp
        in_maps.append(m)
    return in_maps, row_biases_zero, ln_trivial


def kernel(**inputs) -> np.ndarray:
    in_maps, row_biases_zero, ln_trivial = _prep_in_maps(inputs)
    nc = _get_nc(row_biases_zero, ln_trivial)
    # The axon/NRT path occasionally throws a transient
    # NRT_EXEC_UNIT_UNRECOVERABLE on dispatch; the devices recover, so retry.
    last_err = None
    for attempt in range(3):
        try:
            res = run_bass_kernel_spmd(nc, in_maps, core_ids=list(range(N)))
            break
        except Exception as e:  # noqa: BLE001
            last_err = e
            import time as _time

            _time.sleep(5.0)
    else:
        raise last_err
    out = np.empty((N, T, V), np.float32)
    for i in range(N):
        out[i] = np.asarray(res.results[i]["logits"]).astype(np.float32)
    return out



# revision 64
# speedup vs baseline: 1.0010x; 1.0010x over previous
"""Trainium2 Bass kernel for nn_CaptioningTransformer.

Data-parallel over batch N=8 across the 8 NeuronCores (one caption per core).
Each core runs the full 2-layer decoder + the (512,512)@(512,32000) logits
projection for its caption. Layer matmuls run in bf16 (fp32 PSUM
accumulation); LayerNorm / softmax statistics / residual stream stay fp32.

The logits projection runs in fp8e4 DoubleRow mode with a 3-term hi/lo
decomposition: x ~ (xh + xl)/SX, w ~ (wh + wl)/SW, and
x@w ~ (xh@wh + xh@wl + xl@wh) / (SX*SW), all three products accumulated in
one PSUM group (shared scale). Residuals stay in e4m3 normal range thanks to
the large base scales, so precision matches bf16. Logits are written to HBM
as bf16 and upcast on the host.

Self-contained: hardcodes all shapes; takes FULL inputs, returns FULL output.
"""

import math
from contextlib import ExitStack

import ml_dtypes
import numpy as np

import concourse.bacc as bacc
import concourse.bass as bass
import concourse.tile as tile
from concourse import mybir
from concourse.bass_utils import run_bass_kernel_spmd
from concourse.masks import make_causal_mask, make_identity

# dims
N, T, D, W, H, V, L, FF = 8, 512, 1024, 512, 4, 32000, 2, 2048
P = 128
TC = T // P            # 4 token chunks
KC = W // P            # 4 feature chunks
DC = D // P            # 8 vis-feature chunks
FFC = FF // P          # 16 ffn chunks
HD = W // H            # 128 head dim (== P)
VG = 2000              # vocab columns per DMA group
NVG = V // VG          # 16 groups
SV = 500               # vocab columns per psum tile
NSV = VG // SV         # 4 subtiles per group
EPS = 1e-5
SCALE = 1.0 / math.sqrt(HD)
CPACK_COLS = 4 + DC + 4 * L + 4 * L + 4 * L + FFC * L + W * L

F32 = mybir.dt.float32
BF16 = mybir.dt.bfloat16
F8 = mybir.dt.float8e4
I32 = mybir.dt.int32
AX = mybir.AxisListType
ALU = mybir.AluOpType
ACTF = mybir.ActivationFunctionType
PERF2 = mybir.MatmulPerfMode.DoubleRow
BF16_NP = ml_dtypes.bfloat16
F8_NP = ml_dtypes.float8_e4m3

# fp8 hi/lo base scales for the logits matmul (shared-scale 3-term scheme)
SX = 32.0
SW = 512.0
OSCALE = 1.0 / (SX * SW)


def _wrap_p(a, np_dtype):
    """[..., k*P, X] -> [..., P, k, X] (partition-major wrap of the -2 axis)."""
    a = np.asarray(a)
    lead = a.shape[:-2]
    k = a.shape[-2] // P
    x = a.shape[-1]
    a = a.reshape(*lead, k, P, x)
    a = np.moveaxis(a, -2, -3)  # [..., P, k, x]
    return np.ascontiguousarray(a.astype(np_dtype))


def _wrap_vec(v, np_dtype):
    """[..., k*P] -> [..., P, k]."""
    v = np.asarray(v)
    lead = v.shape[:-1]
    k = v.shape[-1] // P
    v = v.reshape(*lead, k, P)
    v = np.moveaxis(v, -1, -2)
    return np.ascontiguousarray(v.astype(np_dtype))


def _build(row_biases_zero: bool, ln_trivial: bool, stop_after: str | None = None):
    nc = bacc.Bacc(
        "TRN2", target_bir_lowering=False, debug=False, enable_asserts=False
    )

    def din(name, shape, dt):
        return nc.dram_tensor(name, list(shape), dt, kind="ExternalInput").ap()

    # ---- DRAM inputs (per core) ----
    x0_d = din("x0", [P, TC, W], F32)             # emb[captions] + pe (host)
    visw_d = din("visw", [P, DC, W], BF16)
    # packed f32 consts: visb(4) feat(8) sabq(2*4) sabk(2*4) cabv(2*4)
    # ff1b(2*16) then cabo rows (row 0 only, 2*512)
    cpack_d = din("cpack", [P, CPACK_COLS], F32)
    sa_d = din("sa", [L, P, 4, KC, W], BF16)      # q,k,v,o packed
    ca_d = din("ca", [L, P, 2, KC, W], BF16)      # wv,wo packed
    ff_d = din("ff", [L, P, 2, KC * FF], BF16)    # ff1 flat, ff2 flat
    whi_d = din("whi", [P, KC, V], F8)            # e4m3(out_w * SW) hi part
    wlo_d = din("wlo", [P, KC, V], F8)            # e4m3 residual (same scale)
    if not row_biases_zero:
        sabv_d = din("sabv", [L, 1, W], BF16)
        sabo_d = din("sabo", [L, 1, W], BF16)
        ff2b_d = din("ff2b", [L, 1, W], BF16)
        outb_d = din("outb", [1, V], BF16)
    if not ln_trivial:
        lnw_d = [din(f"ln{i}w", [L, 1, W], F32) for i in (1, 2, 3)]
        lnb_d = [din(f"ln{i}b", [L, 1, W], F32) for i in (1, 2, 3)]

    out_d = nc.dram_tensor("logits", [T, V], BF16, kind="ExternalOutput").ap()

    with tile.TileContext(nc) as tc, ExitStack() as ctx:
        consts = ctx.enter_context(tc.tile_pool(name="consts", bufs=1))
        xpool = ctx.enter_context(tc.tile_pool(name="xpool", bufs=1))
        wpool = ctx.enter_context(tc.tile_pool(name="wpool", bufs=1))
        work = ctx.enter_context(tc.tile_pool(name="work", bufs=1))
        hot = ctx.enter_context(tc.tile_pool(name="hot", bufs=3))
        sap = ctx.enter_context(tc.tile_pool(name="sap", bufs=2))
        # fallback variants carry extra bias/LN tiles; shrink the perf-only
        # rings there (that path's speed is irrelevant)
        _fast = row_biases_zero and ln_trivial
        wpre = ctx.enter_context(tc.tile_pool(name="wpre", bufs=2 if _fast else 1))
        ostp = ctx.enter_context(tc.tile_pool(name="ostp", bufs=6 if _fast else 2))
        if not ln_trivial:
            lnp = ctx.enter_context(tc.tile_pool(name="lnp", bufs=1))
        psA = ctx.enter_context(tc.tile_pool(name="psA", bufs=2, space="PSUM"))
        psS = ctx.enter_context(tc.tile_pool(name="psS", bufs=3, space="PSUM"))
        psT = ctx.enter_context(tc.tile_pool(name="psT", bufs=3, space="PSUM"))

        # ---- residual stream: x0 = emb[captions] + pe, gathered host-side;
        # loaded per chunk so the first transpose starts as early as possible
        x_sb = xpool.tile([P, TC, W], F32)
        for c in range(TC):
            nc.sync.dma_start(x_sb[:, c], x0_d[:, c])

        # ---- constants ----
        ident_f32 = consts.tile([P, P], F32)
        make_identity(nc, ident_f32[:])
        causalT = consts.tile([P, P], F32)
        nc.gpsimd.memset(causalT[:], 0.0)
        nc.gpsimd.affine_select(
            out=causalT[:], in_=causalT[:], compare_op=ALU.is_ge,
            fill=-1e9, base=0, pattern=[[1, P]], channel_multiplier=-1,
        )
        ones_col_bf = consts.tile([P, 1], BF16)
        nc.vector.memset(ones_col_bf[:], 1.0)
        ones_bf = consts.tile([1, P], BF16)
        nc.vector.memset(ones_bf[:], 1.0)
        ones_f32 = consts.tile([1, P], F32)
        nc.vector.memset(ones_f32[:], 1.0)
        eps_sb = consts.tile([P, 1], F32)
        nc.vector.memset(eps_sb[:], EPS)

        cpack_sb = consts.tile([P, CPACK_COLS], F32)
        nc.sync.dma_start(cpack_sb[:], cpack_d[:])
        o = 0
        visb_sb = cpack_sb[:, o : o + KC]; o += KC
        feat_sb = cpack_sb[:, o : o + DC]; o += DC
        sabq_sb = [cpack_sb[:, o + 4 * l : o + 4 * (l + 1)] for l in range(L)]
        o += 4 * L
        sabk_sb = [cpack_sb[:, o + 4 * l : o + 4 * (l + 1)] for l in range(L)]
        o += 4 * L
        cabv_sb = [cpack_sb[:, o + 4 * l : o + 4 * (l + 1)] for l in range(L)]
        o += 4 * L
        ff1b_sb = [cpack_sb[:, o + FFC * l : o + FFC * (l + 1)] for l in range(L)]
        o += FFC * L
        cabo_sb = [cpack_sb[0:1, o + W * l : o + W * (l + 1)] for l in range(L)]
        o += W * L
        featb_sb = consts.tile([P, DC], BF16)
        nc.vector.tensor_copy(featb_sb[:], feat_sb)

        def per_layer_rows(dram, nm, dt, shape):
            tiles = []
            for l in range(L):
                t = consts.tile(shape, dt, name=f"{nm}{l}")
                nc.sync.dma_start(t[:], dram[l])
                tiles.append(t)
            return tiles
        if not row_biases_zero:
            sabv_sb = per_layer_rows(sabv_d, "sabv", BF16, [1, W])
            sabo_sb = per_layer_rows(sabo_d, "sabo", BF16, [1, W])
            ff2b_sb = per_layer_rows(ff2b_d, "ff2b", BF16, [1, W])
            # out_b enters the fp8 logits PSUM at the shared pre-scale
            ones16k_bf = consts.tile([1, P], BF16)
            nc.vector.memset(ones16k_bf[:], 1.0 / OSCALE)
        if not ln_trivial:
            # broadcast ln scale/bias rows across partitions once
            lnw_bc = [[None] * L for _ in range(3)]
            lnb_bc = [[None] * L for _ in range(3)]
            for i in range(3):
                for l in range(L):
                    wt = lnp.tile([P, W], F32, name=f"lnwbc{i}_{l}", tag=f"lnw{i}", bufs=1)
                    nc.gpsimd.dma_start(wt[:], lnw_d[i][l].to_broadcast([P, W]))
                    lnw_bc[i][l] = wt
                    bt = lnp.tile([P, W], F32, name=f"lnbbc{i}_{l}", tag=f"lnb{i}", bufs=1)
                    nc.gpsimd.dma_start(bt[:], lnb_d[i][l].to_broadcast([P, W]))
                    lnb_bc[i][l] = bt

        # ---- layer-0 self-attention weights; q,k first (critical path) ----
        sa0_sb = sap.tile([P, 4, KC, W], BF16, name="sa_sb", tag="sa_sb")
        for q in range(2):
            nc.sync.dma_start(sa0_sb[:, q], sa_d[0, :, q])

        # ---- vis projection weights ----
        visw_sb = work.tile([P, DC, W], BF16, name="visw_sb", tag="hT")
        nc.sync.dma_start(visw_sb[:], visw_d[:])
        for q in range(2, 4):
            nc.sync.dma_start(sa0_sb[:, q], sa_d[0, :, q])

        _stages = {
            "embed": 0, "memT": 1, "sa0": 2, "ca0": 3, "l0": 4, "l1": 5,
            "logits1": 6, None: 99,
        }
        srank = _stages[stop_after]

        # cross-attention weights: loaded upfront (DMA overlaps layer-0 SA),
        # consumed by the deferred precompute below
        ca_sb_tiles = []
        if srank >= 3:
            for l in range(L):
                cal_sb = wpool.tile([P, 2, KC, W], BF16, name=f"ca{l}", tag=f"ca{l}")
                nc.sync.dma_start(cal_sb[:, 0], ca_d[l, :, 0])
                nc.sync.dma_start(cal_sb[:, 1], ca_d[l, :, 1])
                ca_sb_tiles.append(cal_sb)

        # ---- memory vector memT = (features @ vis_w + vis_b), transposed [W,1]
        # Runs between layer-0 SA and the first ca-add (off the startup
        # critical path; the PE is otherwise busy with attention by then).
        memT_sb = consts.tile([P, KC], BF16)

        def precompute_memT():
            for o in range(KC):
                pm = psS.tile([P, 512], F32, name="psS", tag="psS")
                for ki in range(DC):
                    nc.tensor.matmul(
                        pm[:, :1],
                        lhsT=visw_sb[:, ki, o * P : (o + 1) * P],
                        rhs=featb_sb[:, ki : ki + 1],
                        start=(ki == 0),
                        stop=(ki == DC - 1),
                    )
                nc.scalar.activation(
                    memT_sb[:, o : o + 1], pm[:, :1], ACTF.Identity,
                    bias=visb_sb[:, o : o + 1], scale=1.0,
                )

        # ---- cross-attention rows (x-independent: softmax over single key
        # is identically 1, so ca_out = (mem@wv+bv)@wo+bo broadcast over T).
        ca_bc = []

        def precompute_ca():
            precompute_memT()
            for l in range(L):
                cawv_sb, cawo_sb = ca_sb_tiles[l][:, 0], ca_sb_tiles[l][:, 1]
                vTca = hot.tile([P, KC], BF16, name="vTca", tag="vTca")
                for o in range(KC):
                    pm = psS.tile([P, 512], F32, name="psS", tag="psS")
                    for ki in range(KC):
                        nc.tensor.matmul(
                            pm[:, :1],
                            lhsT=cawv_sb[:, ki, o * P : (o + 1) * P],
                            rhs=memT_sb[:, ki : ki + 1],
                            start=(ki == 0),
                            stop=(ki == KC - 1),
                        )
                    nc.scalar.activation(
                        vTca[:, o : o + 1], pm[:, :1], ACTF.Identity,
                        bias=cabv_sb[l][:, o : o + 1], scale=1.0,
                    )
                pr = psS.tile([P, 512], F32, name="psS", tag="psS")
                for o in range(KC):
                    nc.tensor.matmul(
                        pr[:1, :],
                        lhsT=vTca[:, o : o + 1],
                        rhs=cawo_sb[:, o, :],
                        start=(o == 0),
                        stop=(o == KC - 1),
                    )
                ca_row = hot.tile([1, W], F32, name="ca_row", tag="ca_row", bufs=1)
                nc.vector.tensor_tensor(
                    ca_row[:], pr[:1, :], cabo_sb[l], op=ALU.add
                )
                pbc = psS.tile([P, 512], F32, name="psS", tag="psS")
                nc.tensor.matmul(
                    pbc[:], lhsT=ones_f32[:], rhs=ca_row[:], start=True, stop=True
                )
                cb = consts.tile([P, W], F32, name=f"ca_bc{l}")
                nc.scalar.copy(cb[:], pbc[:])
                ca_bc.append(cb)

        def ln_chunk(ln_idx, l, c):
            """x_sb[:, c] <- LN(x_sb[:, c]) (free-axis stats)."""
            if True:
                stats = hot.tile([P, 6], F32, name="lnstats", tag="lnstats")
                nc.vector.bn_stats(stats[:], x_sb[:, c, :])
                mv = hot.tile([P, 2], F32, name="lnmv", tag="lnmv")
                nc.vector.bn_aggr(mv[:], stats[:])
                std = hot.tile([P, 1], F32, name="lnstd", tag="lnstd")
                nc.scalar.activation(
                    std[:], mv[:, 1:2], ACTF.Sqrt, bias=eps_sb[:], scale=1.0
                )
                rstd = hot.tile([P, 1], F32, name="lnrstd", tag="lnrstd")
                nc.vector.reciprocal(rstd[:], std[:])
                nmr = hot.tile([P, 1], F32, name="lnnmr", tag="lnnmr")
                nc.vector.scalar_tensor_tensor(
                    nmr[:], mv[:, 0:1], -1.0, rstd[:],
                    op0=ALU.mult, op1=ALU.mult,
                )
                nc.scalar.activation(
                    x_sb[:, c, :], x_sb[:, c, :], ACTF.Identity,
                    bias=nmr[:], scale=rstd[:],
                )
                if not ln_trivial:
                    nc.vector.tensor_tensor(
                        x_sb[:, c, :], x_sb[:, c, :], lnw_bc[ln_idx][l][:],
                        op=ALU.mult,
                    )
                    nc.vector.tensor_tensor(
                        x_sb[:, c, :], x_sb[:, c, :], lnb_bc[ln_idx][l][:],
                        op=ALU.add,
                    )

        def transpose_chunk(xt_tile, c):
            """xt_tile[p, o, c*P:(c+1)*P] <- x_sb[:, c, :].T (bf16)"""
            for o in range(KC):
                pt = psT.tile([P, P], F32, name="ptr", tag="ptr")
                nc.tensor.transpose(
                    pt[:], x_sb[:, c, o * P : (o + 1) * P], ident_f32[:]
                )
                if (c + o) % 2 == 0:
                    nc.vector.tensor_copy(
                        xt_tile[:, o, c * P : (c + 1) * P], pt[:]
                    )
                else:
                    nc.scalar.copy(xt_tile[:, o, c * P : (c + 1) * P], pt[:])

        def transpose_x_to(xt_tile):
            """xt_tile[p, o, t] (bf16) <- x_sb[t%P, t//P, o*P+p]"""
            for c in range(TC):
                for o in range(KC):
                    pt = psT.tile([P, P], F32, name="ptr", tag="ptr")
                    nc.tensor.transpose(
                        pt[:], x_sb[:, c, o * P : (o + 1) * P], ident_f32[:]
                    )
                    if (c + o) % 2 == 0:
                        nc.vector.tensor_copy(
                            xt_tile[:, o, c * P : (c + 1) * P], pt[:]
                        )
                    else:
                        nc.scalar.copy(xt_tile[:, o, c * P : (c + 1) * P], pt[:])

        # logits-weight prefetch ring: [hi, lo] pair per vocab group, issued
        # early on the SP queue so transfers fill DMA slack under layer compute
        wl_tiles = {}

        def prefetch_wl(vg, eng=None):
            if vg >= NVG or srank < 5 or vg in wl_tiles:
                return
            # rotate through the wpre ring plus the sa/ff weight buffers
            # (dead after their layer) for extra prefetch depth
            slot = vg % 4 if vg >= 4 else vg
            if slot < 2:
                t = wpre.tile([P, 2, KC, VG], F8, name="wl", tag="wl")
            elif slot == 2:
                t = sap.tile([P, 2, KC, VG], F8, name="wl", tag="sa_sb")
            else:
                t = wpool.tile([P, 2, KC, VG], F8, name="wl", tag="ff_sb")
            # layer-phase prefetches issue on SP (ordered after the layer
            # weight loads); steady-state ones on the idle Pool queue so the
            # ring-slot wait does not block SP's latency-critical ost writes
            eng = eng or nc.gpsimd
            eng.dma_start(t[:, 0], whi_d[:, :, vg * VG : (vg + 1) * VG])
            eng.dma_start(t[:, 1], wlo_d[:, :, vg * VG : (vg + 1) * VG])
            wl_tiles[vg] = t

        # logits operand: xh8 = e4m3(SX*x), xl8 = e4m3(SX*x - xh8), [W, T].
        # Produced chunk-by-chunk inside the final layer norm.
        xh8 = work.tile([P, KC, T], F8, name="xh8", tag="qT")
        xl8 = work.tile([P, KC, T], F8, name="xl8", tag="kT")

        def convert_chunk(c):
            for o in range(KC):
                pt = psT.tile([P, P], F32, name="ptr", tag="ptr")
                nc.tensor.transpose(
                    pt[:], x_sb[:, c, o * P : (o + 1) * P], ident_f32[:]
                )
                hs = xh8[:, o, c * P : (c + 1) * P]
                nc.scalar.activation(hs, pt[:], ACTF.Copy, scale=SX)
                nc.vector.scalar_tensor_tensor(
                    xl8[:, o, c * P : (c + 1) * P], pt[:], SX, hs,
                    op0=ALU.mult, op1=ALU.subtract,
                )

        # ================= layers =================
        for l in range(L if srank >= 2 else 0):
            # ---- self attention ----
            if l == 0:
                sal_sb = sa0_sb
            else:
                sal_sb = sap.tile([P, 4, KC, W], BF16, name="sa_sb", tag="sa_sb")
                for q in range(4):
                    nc.sync.dma_start(sal_sb[:, q], sa_d[l, :, q])
                prefetch_wl(2, nc.sync)
            saq_sb, sak_sb = sal_sb[:, 0], sal_sb[:, 1]
            sav_sb, sao_sb = sal_sb[:, 2], sal_sb[:, 3]

            if l == 0:
                xT = work.tile([P, KC, T], BF16, name="xT", tag="xT")
                transpose_x_to(xT)
            else:
                xT = xT_next

            qT = work.tile([P, KC, T], BF16, name="qT", tag="qT")
            kT = work.tile([P, KC, T], BF16, name="kT", tag="kT")
            qk_cw = P
            for dst, wsb, bsb in ((qT, saq_sb, sabq_sb[l]), (kT, sak_sb, sabk_sb[l])):
                for o in range(KC):
                    pq = psA.tile([P, 512], F32, name="psA", tag="psA")
                    for t0 in range(0, T, qk_cw):
                        for ki in range(KC):
                            nc.tensor.matmul(
                                pq[:, t0 : t0 + qk_cw],
                                lhsT=wsb[:, ki, o * P : (o + 1) * P],
                                rhs=xT[:, ki, t0 : t0 + qk_cw],
                                start=(ki == 0),
                                stop=(ki == KC - 1),
                            )
                    nc.scalar.activation(
                        dst[:, o, :], pq[:], ACTF.Identity,
                        bias=bsb[:, o : o + 1], scale=1.0,
                    )
            v_sb = work.tile([P, TC, W], BF16, name="v_sb", tag="v_sb")
            for c in range(TC):
                pv = psA.tile([P, 512], F32, name="psA", tag="psA")
                first = True
                if not row_biases_zero:
                    nc.tensor.matmul(
                        pv[:], lhsT=ones_bf[:], rhs=sabv_sb[l][:],
                        start=True, stop=False,
                    )
                    first = False
                for ki in range(KC):
                    nc.tensor.matmul(
                        pv[:],
                        lhsT=xT[:, ki, c * P : (c + 1) * P],
                        rhs=sav_sb[:, ki, :],
                        start=first,
                        stop=(ki == KC - 1),
                    )
                    first = False
                if c % 2 == 0:
                    nc.vector.tensor_copy(v_sb[:, c, :], pv[:])
                else:
                    nc.scalar.copy(v_sb[:, c, :], pv[:])

            yT = work.tile([P, H, T], BF16, name="yT", tag="yT")
            for h in range(H):
                # scores computed pre-transposed [tk, tq] (swap q/k roles), so
                # exp() writes the A@V operand directly -- no PE transposes.
                # Probs stay UNNORMALIZED (exp can't overflow at these scales);
                # normalization is applied per-head at the out-projection.
                AT = work.tile([P, TC, T], BF16, name="AT", tag="AT", bufs=2)
                for j in range(TC):
                    nv = T - j * P  # valid tq suffix for tk-chunk j
                    ps = psS.tile([P, 512], F32, name="psS", tag="psS")
                    nc.tensor.matmul(
                        ps[:, :nv],
                        lhsT=kT[:, h, j * P : (j + 1) * P],
                        rhs=qT[:, h, j * P :],
                        start=True,
                        stop=True,
                    )
                    # additive -1e9 strict lower-triangle on the diagonal block
                    nc.vector.tensor_tensor(
                        ps[:, :P], ps[:, :P], causalT[:], op=ALU.add
                    )
                    nc.scalar.activation(
                        AT[:, j, j * P :], ps[:, :nv], ACTF.Exp,
                        bias=0.0, scale=SCALE,
                    )
                # per-tq row sums of the unnormalized probs as a [1, T] row
                # (ones^T @ AT); the reciprocal row is broadcast back over
                # partitions with one ones-matmul, overlapping the AV matmuls
                rsum = psT.tile([P, 512], F32, name="rsum", tag="ptr")
                for j in range(TC):
                    nc.tensor.matmul(
                        rsum[:1, j * P :],
                        lhsT=ones_col_bf[:],
                        rhs=AT[:, j, j * P :],
                        start=(j == 0),
                        stop=(j == TC - 1),
                    )
                rrow = hot.tile([1, T], BF16, name="rrow", tag="rrow")
                with nc.allow_low_precision("probs are unnormalized O(1)"):
                    nc.vector.reciprocal(rrow[:], rsum[:1, :])
                py = psA.tile([P, 512], F32, name="psY", tag="psA")
                for j in range(TC):
                    nc.tensor.matmul(
                        py[:, j * P :],
                        lhsT=v_sb[:, j, h * HD : (h + 1) * HD],
                        rhs=AT[:, j, j * P :],
                        start=(j == 0),
                        stop=(j == TC - 1),
                    )
                # pb[d, tok] = 1/rowsum broadcast over the d-partitions;
                # yT = py * pb (normalized here so the out-projection can
                # accumulate all heads in one PSUM group)
                pb = psA.tile([P, 512], F32, name="pb", tag="psA")
                nc.tensor.matmul(
                    pb[:], lhsT=ones_bf[:], rhs=rrow[:], start=True, stop=True
                )
                # DVE can read only one PSUM operand; stage pb through SBUF
                pb_sb = hot.tile([P, T], BF16, name="pb_sb", tag="pb_sb", bufs=2)
                nc.scalar.copy(pb_sb[:], pb[:])
                nc.vector.tensor_tensor(yT[:, h, :], py[:], pb_sb[:], op=ALU.mult)

            # per-head out projection; normalization folded into the
            # per-partition scale of the fused residual accumulate
            if l == 0 and srank >= 3:
                precompute_ca()
            if srank >= 4:
                xT2 = work.tile([P, KC, T], BF16, name="xT2", tag="xT")
            for c in range(TC):
                po = psT.tile([P, 512], F32, name="po", tag="ptr")
                for h in range(H):
                    nc.tensor.matmul(
                        po[:],
                        lhsT=yT[:, h, c * P : (c + 1) * P],
                        rhs=sao_sb[:, h, :],
                        start=(h == 0),
                        stop=(h == H - 1),
                    )
                nc.vector.tensor_add(x_sb[:, c, :], x_sb[:, c, :], po[:])
                if not row_biases_zero:
                    pob = psS.tile([P, 512], F32, name="psS", tag="psS")
                    nc.tensor.matmul(
                        pob[:], lhsT=ones_bf[:], rhs=sabo_sb[l][:],
                        start=True, stop=True,
                    )
                    nc.vector.tensor_add(x_sb[:, c, :], x_sb[:, c, :], pob[:])
                ln_chunk(0, l, c)
            if srank == 2:
                break
            for c in range(TC):
                nc.vector.tensor_add(x_sb[:, c, :], x_sb[:, c, :], ca_bc[l][:])
                ln_chunk(1, l, c)
            for c in range(TC):
                if srank >= 4:
                    transpose_chunk(xT2, c)
            if srank == 3:
                break

            # ---- ffn ----
            ffl_sb = wpool.tile([P, 2, KC * FF], BF16, name="ff_sb", tag="ff_sb")
            nc.sync.dma_start(ffl_sb[:, 0], ff_d[l, :, 0])
            nc.sync.dma_start(ffl_sb[:, 1], ff_d[l, :, 1])
            if l == 0:
                for pv in range(2):
                    prefetch_wl(pv, nc.sync)
            else:
                prefetch_wl(3, nc.sync)
            ff1_sb = ffl_sb[:, 0].rearrange("p (k f) -> p k f", k=KC)
            ff2_sb = ffl_sb[:, 1].rearrange("p (m w) -> p m w", m=FFC)

            hT = work.tile([P, FFC, T], BF16, name="hT", tag="hT")
            for m in range(FFC):
                ph = psA.tile([P, 512], F32, name="psA", tag="psA")
                for t0 in range(0, T, P):
                    for ki in range(KC):
                        nc.tensor.matmul(
                            ph[:, t0 : t0 + P],
                            lhsT=ff1_sb[:, ki, m * P : (m + 1) * P],
                            rhs=xT2[:, ki, t0 : t0 + P],
                            start=(ki == 0),
                            stop=(ki == KC - 1),
                        )
                nc.scalar.activation(
                    hT[:, m, :], ph[:], ACTF.Relu,
                    bias=ff1b_sb[l][:, m : m + 1], scale=1.0,
                )
            if l < L - 1:
                xT_next = work.tile([P, KC, T], BF16, name="xT", tag="xT")
            for c in range(TC):
                pf2 = psA.tile([P, 512], F32, name="psA", tag="psA")
                first = True
                if not row_biases_zero:
                    nc.tensor.matmul(
                        pf2[:], lhsT=ones_bf[:], rhs=ff2b_sb[l][:],
                        start=True, stop=False,
                    )
                    first = False
                for m in range(FFC):
                    nc.tensor.matmul(
                        pf2[:],
                        lhsT=hT[:, m, c * P : (c + 1) * P],
                        rhs=ff2_sb[:, m, :],
                        start=first,
                        stop=(m == FFC - 1),
                    )
                    first = False
                nc.vector.tensor_add(x_sb[:, c, :], x_sb[:, c, :], pf2[:])
                ln_chunk(2, l, c)
                if l == L - 1 and srank >= 5:
                    convert_chunk(c)
                elif l < L - 1:
                    transpose_chunk(xT_next, c)
            if srank == 4:
                break

        # ================= logits (fp8 3-term DoubleRow) =================
        _nvg = NVG if srank >= 99 else (1 if srank >= 6 else 0)
        pcnt = 0
        for vg in range(_nvg):
            if vg not in wl_tiles:
                prefetch_wl(vg)
            wl = wl_tiles.pop(vg)
            if not row_biases_zero:
                ob_sb = ostp.tile([1, VG], BF16, name="ob", tag="ob", bufs=2)
                nc.sync.dma_start(ob_sb[:], outb_d[:, vg * VG : (vg + 1) * VG])
            for c in range(TC):
                ost = ostp.tile([P, VG], BF16, name="ost", tag="ost")
                for sv in range(NSV):
                    plp = (psA, psS, psT)[pcnt % 3]
                    pcnt += 1
                    ptag = "ptr" if plp is psT else plp.name
                    pl = plp.tile([P, 512], F32, name="psL", tag=ptag)
                    first = True
                    if not row_biases_zero:
                        nc.tensor.matmul(
                            pl[:, :SV],
                            lhsT=ones16k_bf[:],
                            rhs=ob_sb[:, sv * SV : (sv + 1) * SV],
                            start=True,
                            stop=False,
                            skip_group_check=True,
                        )
                        first = False
                    svw = slice(sv * SV, (sv + 1) * SV)
                    for xt, wt in ((xh8, 0), (xh8, 1), (xl8, 0)):
                        for kk in (0, 2):
                            nc.tensor.matmul(
                                pl[:, :SV],
                                lhsT=xt[:, kk : kk + 2, c * P : (c + 1) * P],
                                rhs=wl[:, wt, kk : kk + 2, svw],
                                start=first,
                                stop=(xt is xl8 and kk == 2),
                                perf_mode=PERF2,
                                skip_group_check=not row_biases_zero,
                            )
                            first = False
                    if pcnt % 2 == 0:
                        nc.vector.tensor_scalar_mul(ost[:, svw], pl[:, :SV], OSCALE)
                    else:
                        nc.scalar.activation(
                            ost[:, svw], pl[:, :SV], ACTF.Copy, scale=OSCALE
                        )
                nc.sync.dma_start(
                    out_d[c * P : (c + 1) * P, vg * VG : (vg + 1) * VG], ost[:]
                )
            prefetch_wl(vg + 4)

        if stop_after is not None:
            xdbg = nc.dram_tensor(
                "xdbg", [P, TC, W], F32, kind="ExternalOutput"
            ).ap()
            nc.sync.dma_start(xdbg[:], x_sb[:])

    nc.compile()
    return nc


_BUILD_CACHE = {}


def _get_nc(row_biases_zero, ln_trivial):
    key = (row_biases_zero, ln_trivial)
    if key not in _BUILD_CACHE:
        _BUILD_CACHE[key] = _build(*key)
    return _BUILD_CACHE[key]


def _prep_in_maps(inputs):
    f32 = np.float32
    features = np.asarray(inputs["features"], f32)          # (N, D)
    captions = np.asarray(inputs["captions"])               # (N, T) int
    emb = np.asarray(inputs["emb"], f32)                    # (V, W)
    pe = np.asarray(inputs["pe"], f32)                      # (T, W)
    x0 = emb[captions] + pe[None]                           # (N, T, W)

    row_biases_zero = all(
        not np.any(np.asarray(inputs[k]))
        for k in ("sa_bv", "sa_bo", "ff2_b", "out_b")
    )
    ln_trivial = all(
        np.all(np.asarray(inputs[f"ln{i}_w"]) == 1.0)
        and not np.any(np.asarray(inputs[f"ln{i}_b"]))
        for i in (1, 2, 3)
    )

    sa_pack = np.stack(
        [_wrap_p(np.asarray(inputs[k]), BF16_NP)
         for k in ("sa_wq", "sa_wk", "sa_wv", "sa_wo")], axis=1
    )  # [L, 4, P, KC, W] -> want [L, P, 4, KC, W]
    sa_pack = np.ascontiguousarray(np.moveaxis(sa_pack, 1, 2))
    ca_pack = np.stack(
        [_wrap_p(np.asarray(inputs[k]), BF16_NP) for k in ("ca_wv", "ca_wo")],
        axis=1,
    )
    ca_pack = np.ascontiguousarray(np.moveaxis(ca_pack, 1, 2))
    ff1w = _wrap_p(np.asarray(inputs["ff1_w"]), BF16_NP).reshape(L, P, KC * FF)
    ff2w = _wrap_p(np.asarray(inputs["ff2_w"]), BF16_NP).reshape(L, P, FFC * W)
    ff_pack = np.ascontiguousarray(np.stack([ff1w, ff2w], axis=2))  # [L,P,2,KF]

    cpack = np.zeros((P, CPACK_COLS), f32)
    o = 0
    cpack[:, o : o + KC] = _wrap_vec(np.asarray(inputs["vis_b"]), f32); o += KC
    feat_off = o; o += DC  # per-core features slot
    sabq = _wrap_vec(np.asarray(inputs["sa_bq"]), f32)
    sabk = _wrap_vec(np.asarray(inputs["sa_bk"]), f32)
    cabv = _wrap_vec(np.asarray(inputs["ca_bv"]), f32)
    ff1b = _wrap_vec(np.asarray(inputs["ff1_b"]), f32)
    cabo = np.asarray(inputs["ca_bo"], f32)
    for l in range(L):
        cpack[:, o + 4 * l : o + 4 * (l + 1)] = sabq[l]
    o += 4 * L
    for l in range(L):
        cpack[:, o + 4 * l : o + 4 * (l + 1)] = sabk[l]
    o += 4 * L
    for l in range(L):
        cpack[:, o + 4 * l : o + 4 * (l + 1)] = cabv[l]
    o += 4 * L
    for l in range(L):
        cpack[:, o + FFC * l : o + FFC * (l + 1)] = ff1b[l]
    o += FFC * L
    for l in range(L):
        cpack[0, o + W * l : o + W * (l + 1)] = cabo[l]
    o += W * L
    assert o == CPACK_COLS

    # fp8 hi/lo split of out_w at shared scale SW; [W, V] -> [P, KC, V]
    w_s = np.asarray(inputs["out_w"], f32) * SW
    whi = np.clip(w_s, -240.0, 240.0).astype(F8_NP)
    wlo = np.clip(w_s - whi.astype(f32), -240.0, 240.0).astype(F8_NP)

    def _wrap_w(a):
        a = a.reshape(KC, P, V)
        return np.ascontiguousarray(np.moveaxis(a, 0, 1))

    shared = {
        "visw": _wrap_p(np.asarray(inputs["vis_w"]), BF16_NP),
        "sa": sa_pack,
        "ca": ca_pack,
        "ff": ff_pack,
        "whi": _wrap_w(whi),
        "wlo": _wrap_w(wlo),
    }
    if not row_biases_zero:
        shared["sabv"] = np.ascontiguousarray(
            np.asarray(inputs["sa_bv"]).astype(BF16_NP).reshape(L, 1, W)
        )
        shared["sabo"] = np.ascontiguousarray(
            np.asarray(inputs["sa_bo"]).astype(BF16_NP).reshape(L, 1, W)
        )
        shared["ff2b"] = np.ascontiguousarray(
            np.asarray(inputs["ff2_b"]).astype(BF16_NP).reshape(L, 1, W)
        )
        shared["outb"] = np.ascontiguousarray(
            np.asarray(inputs["out_b"]).astype(BF16_NP).reshape(1, V)
        )
    if not ln_trivial:
        for i in (1, 2, 3):
            shared[f"ln{i}w"] = np.ascontiguousarray(
                np.asarray(inputs[f"ln{i}_w"], f32).reshape(L, 1, W)
            )
            shared[f"ln{i}b"] = np.ascontiguousarray(
                np.asarray(inputs[f"ln{i}_b"], f32).reshape(L, 1, W)
            )

    in_maps = []
    for i in range(N):
        m = dict(shared)
        m["x0"] = _wrap_p(x0[i], f32)
        cp = cpack.copy()
        cp[:, feat_off : feat_off + DC] = features[i].reshape(DC, P).T
        m["cpack"] = cp
        in_maps.append(m)
    return in_maps, row_biases_zero, ln_trivial


def kernel(**inputs) -> np.ndarray:
    in_maps, row_biases_zero, ln_trivial = _prep_in_maps(inputs)
    nc = _get_nc(row_biases_zero, ln_trivial)
    # The axon/NRT path occasionally throws a transient
    # NRT_EXEC_UNIT_UNRECOVERABLE on dispatch; the devices recover, so retry.
    last_err = None
    for attempt in range(3):
        try:
            res = run_bass_kernel_spmd(nc, in_maps, core_ids=list(range(N)))
            break
        except Exception as e:  # noqa: BLE001
            last_err = e
            import time as _time

            _time.sleep(5.0)
    else:
        raise last_err
    out = np.empty((N, T, V), np.float32)
    for i in range(N):
        out[i] = np.asarray(res.results[i]["logits"]).astype(np.float32)
    return out

